# revision 62
# baseline (speedup 1.0000x reference)
"""Trainium2 Bass kernel for Convpass-swin hypernet fused adapter.

Reference computation (per batch sample):
  h      = relu(x @ Wm1 + bm1)                    # [B,H,W,64]
  prompt = mean_hw(h) @ Wm2                       # [B,64]  (mean commutes with matmul)
  wflat  = (emb + bm2 + prompt) @ Wh + bh         # [B,96*96*9]
  xd     = quickgelu(x @ Wd + bd)                 # [B,H,W,96]
  y      = quickgelu(conv3x3(xd, wflat))          # per-sample dynamic grouped conv
  out    = y @ Wu + bu                            # [B,H,W,384]

Sharding: data-parallel over batch B=64 across 8 cores (8 samples/core),
weights replicated.

Key layout/precision choices:
- x is shipped pre-transposed ([C, pos] per core) in bf16: no on-device
  transposes, half the DMA bytes.
- The 21MB hypernet matrix Wh streams as fp8(e3m4), scaled by a power of
  two into fp8 range.  The constant part of the conv weights,
  cvec = (emb+bm2)@Wh + bh, is folded in host-side as two extra scaled fp8
  rows (hi + lo residual), so each per-sample conv weight materializes in
  PSUM from a single matmul per 96-column group.
- The 3x3 conv is evaluated tap-major (Wh columns pre-permuted to
  (tap, o, i) order) so the first conv wave overlaps the Wh DMA stream,
  with a one-tap lag so the PSUM->SBUF weight copies never block the PE.
- Conv waves of 2 samples; each wave's matmuls are interleaved with the
  previous wave's output projection so ACT/DVE PSUM-evacuation overlaps
  PE compute.  Output is written bf16 and upcast on the host.

NOTE on DMA ordering: the wu (97-partition bf16) load must not be issued
between the other small weight loads and the x loads — that ordering
triggers a data-corruption bug in the NEFF simulation path (even bf16
elements of later transfers read back as sign*2.0).  wu is issued last
among the constant loads.
"""
import numpy as np
import ml_dtypes

import concourse.bass as bass
import concourse.tile as tile
import concourse.mybir as mybir
from concourse import bacc
from concourse.bass_utils import run_bass_kernel_spmd

F32 = mybir.dt.float32
F32R = mybir.dt.float32r
BF = mybir.dt.bfloat16
FP8 = mybir.dt.float8e3          # e3m4
FP8_NP = ml_dtypes.float8_e3m4
AF = mybir.ActivationFunctionType
AX = mybir.AxisListType
ALU = mybir.AluOpType

# problem constants
B, H, W, C = 64, 28, 28, 384
DIM, E, KK = 96, 64, 3
NCORES = 8
BL = B // NCORES          # samples per core
P = H * W                 # 784 positions per sample
HP = H + 2                # padded spatial
NPOS = BL * P             # 6272 positions per core
NG = DIM * 9              # 864 (tap, o) groups, tap-major
WH_COLS = NG * DIM        # 82944
CHUNK_G = 48              # groups per Wh DMA chunk (= half a tap)
CHUNK_COLS = CHUNK_G * DIM
NCHUNK = NG // CHUNK_G    # 18
WAVES = ((0, 1), (2, 3), (4, 5), (6, 7))


def _round_fp32r(a):
    """Round-to-nearest-even fp32 -> fp32r (11-bit mantissa kept)."""
    b = np.ascontiguousarray(a, np.float32).view(np.uint32).astype(np.uint64)
    bb = b + np.uint64(0x7FF) + ((b >> np.uint64(12)) & np.uint64(1))
    return (bb & np.uint64(0xFFFFF000)).astype(np.uint32).view(np.float32)


def _pow2_scale(a, target=8.0):
    """Power-of-two S with absmax(a)*S <= target."""
    m = float(np.abs(a).max())
    if m == 0.0:
        return 1.0
    return 2.0 ** np.floor(np.log2(target / m))


def build_nc():
    nc = bacc.Bacc("TRN2", target_bir_lowering=False, debug=False)

    x_d = nc.dram_tensor("x", [128, C // 128, NPOS], BF, kind="ExternalInput").ap()
    wm1_d = nc.dram_tensor("wm1", [C, E], BF, kind="ExternalInput").ap()
    wd_d = nc.dram_tensor("wd", [C, DIM], BF, kind="ExternalInput").ap()
    wm2_d = nc.dram_tensor("wm2", [E, E], F32R, kind="ExternalInput").ap()
    bm1_d = nc.dram_tensor("bm1", [E], F32, kind="ExternalInput").ap()
    bd_d = nc.dram_tensor("bd", [DIM], F32, kind="ExternalInput").ap()
    scrow_d = nc.dram_tensor("scrow", [2, 1], F32, kind="ExternalInput").ap()
    wh_d = nc.dram_tensor("wh", [E + 2, WH_COLS], FP8, kind="ExternalInput").ap()
    wu_d = nc.dram_tensor("wu", [DIM + 1, C], BF, kind="ExternalInput").ap()
    out_d = nc.dram_tensor("out", [NPOS, C], BF, kind="ExternalOutput").ap()

    with tile.TileContext(nc) as tc:
        with (
            tc.tile_pool(name="const", bufs=1) as cp,
            tc.tile_pool(name="persist", bufs=1) as pp,
            tc.tile_pool(name="wh", bufs=10) as wh_p,
            tc.tile_pool(name="hscr", bufs=2) as hs_p,
            tc.tile_pool(name="outp", bufs=6) as out_p,
        ):
            # ---- weight loads needed first, then x, then the rest ----
            wm1_sb = cp.tile([128, C // 128, E], BF)
            nc.sync.dma_start(wm1_sb[:], wm1_d.rearrange("(c3 p) e -> p c3 e", p=128))
            wd_sb = cp.tile([128, C // 128, DIM], BF)
            nc.sync.dma_start(wd_sb[:], wd_d.rearrange("(c3 p) e -> p c3 e", p=128))
            wm2_sb = cp.tile([E, E], F32R)
            nc.sync.dma_start(wm2_sb[:], wm2_d[:])
            bm1_sb = cp.tile([E, 1], F32)
            nc.sync.dma_start(bm1_sb[:], bm1_d[:])
            bd_sb = cp.tile([DIM, 1], F32)
            nc.sync.dma_start(bd_sb[:], bd_d[:])
            scrow_sb = cp.tile([2, 1], F32)
            nc.sync.dma_start(scrow_sb[:], scrow_d[:])
            wu_sb = cp.tile([DIM + 1, C], BF)
            nc.sync.dma_start(wu_sb[:], wu_d[:])

            xts = [pp.tile([128, C // 128, P], BF, name=f"xt{b}") for b in range(BL)]
            nc.sync.dma_start(xts[0][:, :, :392], x_d[:, :, 0:392])
            nc.sync.dma_start(xts[0][:, :, 392:], x_d[:, :, 392:P])
            for b in range(1, BL):
                nc.sync.dma_start(xts[b][:], x_d[:, :, b * P:(b + 1) * P])

            # ---- persistent state ----
            xd_pads = [pp.tile([DIM, HP, HP], BF, name=f"xdp{b}") for b in range(BL)]
            w_ts = [pp.tile([DIM, BL, DIM], BF, name=f"wt{t}") for t in range(9)]
            y_bs = [pp.tile([DIM + 1, P], BF, name=f"y{b}") for b in range(BL)]
            hsum = pp.tile([E, BL, 2], F32)
            hbar = pp.tile([E, BL], F32)
            hbar_r = pp.tile([E, BL], F32R)
            pvec = pp.tile([E + 2, BL], BF)

            # Pool-engine inits: conv-pad borders, y ones rows
            for b in range(BL):
                xp = xd_pads[b]
                nc.gpsimd.memset(xp[:, 0:1, :], 0.0)
                nc.gpsimd.memset(xp[:, HP - 1:HP, :], 0.0)
                nc.gpsimd.memset(xp[:, 1:HP - 1, 0:1], 0.0)
                nc.gpsimd.memset(xp[:, 1:HP - 1, HP - 1:HP], 0.0)
                nc.gpsimd.memset(y_bs[b][DIM:DIM + 1, :], 1.0)
            nc.vector.tensor_copy(pvec[E:E + 2, :],
                                  scrow_sb[:, 0:1].to_broadcast((2, BL)))

            # ---- phase 1: per-sample meta sums + xd = gelu(x@Wd + bd) ----
            ps1 = tc.tile_pool(name="ps1", bufs=2, space="PSUM")
            ps1_pool = ps1.__enter__()
            # PE warmup: keep the tensor engine busy through the cost model's
            # clock ramp while the first x tile streams in.
            pwarm = ps1_pool.tile([E, DIM], F32, name="pwarm", tag="warm", bufs=1)
            for w in range(16):
                nc.tensor.matmul(pwarm[:], wm1_sb[:, 0, :], wd_sb[:, 0, :],
                                 start=(w == 0), stop=(w == 15))
            for b in range(BL):
                for h2 in range(2):
                    ph = ps1_pool.tile([E, 392], F32, name="ph", tag="ph")
                    for c in range(C // 128):
                        nc.tensor.matmul(ph[:], wm1_sb[:, c, :],
                                         xts[b][:, c, h2 * 392:(h2 + 1) * 392],
                                         start=(c == 0), stop=(c == 2))
                    if h2 == 0:
                        h_scr = hs_p.tile([E, 392], F32, tag="hscr")
                        nc.scalar.activation(h_scr[:], ph[:], AF.Relu, bias=bm1_sb[:],
                                             accum_out=hsum[:, b, 0:1])
                    else:
                        h_scr = hs_p.tile([E, 392], F32, tag="hscr")
                        nc.vector.tensor_scalar(h_scr[:], ph[:], bm1_sb[:, 0:1], 0.0,
                                                ALU.add, ALU.max)
                        nc.vector.reduce_sum(hsum[:, b, 1:2], h_scr[:], axis=AX.X)
                for h2 in range(2):
                    px = ps1_pool.tile([DIM, 392], F32, name="px", tag="px")
                    for c in range(C // 128):
                        nc.tensor.matmul(px[:], wd_sb[:, c, :],
                                         xts[b][:, c, h2 * 392:(h2 + 1) * 392],
                                         start=(c == 0), stop=(c == 2))
                    nc.scalar.activation(
                        xd_pads[b][:, 1 + h2 * 14: 15 + h2 * 14, 1:29],
                        px[:].rearrange("p (r c) -> p r c", r=14),
                        AF.Gelu_apprx_sigmoid, bias=bd_sb[:])

            # ---- phase 2: prompt -> pvec (scaled by 1/(P*S_wh)) ----
            nc.vector.reduce_sum(hbar[:], hsum[:], axis=AX.X)
            nc.scalar.activation(hbar_r[:], hbar[:], AF.Copy, scale=_HBAR_SCALE[0])
            ppm = ps1_pool.tile([E, BL], F32, name="ppm", tag="ppm", bufs=1)
            nc.tensor.matmul(ppm[:], wm2_sb[:], hbar_r[:], start=True, stop=True)
            nc.scalar.activation(pvec[:E, :], ppm[:], AF.Copy)
            ps1.__exit__(None, None, None)

            def conv_tap(pys_map, wave, t):
                dy, dx = t // 3, t % 3
                for b in wave:
                    for h2 in range(2):
                        nc.tensor.matmul(
                            pys_map[(b, h2)][:], w_ts[t][:, b, :],
                            xd_pads[b][:, h2 * 14 + dy: h2 * 14 + dy + 14,
                                       dx:dx + 28],
                            start=(t == 0), stop=(t == 8))

            def gelu_wave(pys_map, wave):
                for b in wave:
                    for h2 in range(2):
                        nc.scalar.activation(y_bs[b][:DIM, h2 * 392:(h2 + 1) * 392],
                                             pys_map[(b, h2)][:],
                                             AF.Gelu_apprx_sigmoid)

            def out_proj(b, pso):
                ob = out_p.tile([112, 7, C], BF, tag="ob")
                odst = out_d[b * P:(b + 1) * P, :].rearrange("(m p) c -> p m c", p=112)
                for m in range(7):
                    po = pso.tile([112, C], F32, name="po", tag="po")
                    nc.tensor.matmul(po[:], y_bs[b][:, m * 112:(m + 1) * 112], wu_sb[:],
                                     start=True, stop=True)
                    if m % 2 == 0:
                        nc.vector.tensor_copy(ob[:, m, :], po[:])
                    else:
                        nc.scalar.activation(ob[:, m, :], po[:], AF.Copy)
                    if m == 3:
                        nc.sync.dma_start(odst[:, 0:4, :], ob[:, 0:4, :])
                nc.sync.dma_start(odst[:, 4:7, :], ob[:, 4:7, :])

            # ---- phase 3: hypernet streaming + conv wave 0 (one-tap lag) ----
            pyw_ctx = tc.tile_pool(name="pyw0", bufs=1, space="PSUM")
            pyw = pyw_ctx.__enter__()
            pys = {(b, h2): pyw.tile([DIM, 392], F32, name=f"py{b}_{h2}")
                   for b in WAVES[0] for h2 in range(2)}
            psw_ctx = tc.tile_pool(name="psw", bufs=4, space="PSUM")
            psw = psw_ctx.__enter__()
            for j in range(NCHUNK):
                whc = wh_p.tile([E + 2, CHUNK_COLS], FP8, tag="whc")
                nc.sync.dma_start(whc[:], wh_d[:, j * CHUNK_COLS:(j + 1) * CHUNK_COLS])
                t, ohalf = j // 2, j % 2
                pwg = psw.tile([DIM, CHUNK_G, BL], F32, name="pwg", tag="pwg")
                for g in range(CHUNK_G):
                    col = g * DIM
                    nc.tensor.matmul(pwg[:, g, :], whc[:, col:col + DIM], pvec[:],
                                     start=True, stop=True)
                o0 = ohalf * CHUNK_G
                dst = w_ts[t][:, :, o0:o0 + CHUNK_G]
                srcv = pwg[:].rearrange("i g b -> i b g")
                if j % 2 == 0:
                    nc.vector.tensor_copy(dst, srcv)
                else:
                    nc.scalar.activation(dst, srcv, AF.Copy)
                if j >= 3 and j % 2 == 1:
                    conv_tap(pys, WAVES[0], (j - 3) // 2)
            conv_tap(pys, WAVES[0], 8)
            psw_ctx.__exit__(None, None, None)
            gelu_wave(pys, WAVES[0])
            pyw_ctx.__exit__(None, None, None)

            # ---- phases 4+5: conv waves 1..3 overlapped with out-projection ----
            pso_ctx = tc.tile_pool(name="pso", bufs=4, space="PSUM")
            pso = pso_ctx.__enter__()
            prev = WAVES[0]
            for wave in WAVES[1:]:
                pywn_ctx = tc.tile_pool(name=f"pyw{wave[0]}", bufs=1, space="PSUM")
                pywn = pywn_ctx.__enter__()
                pysn = {(b, h2): pywn.tile([DIM, 392], F32, name=f"py{b}_{h2}")
                        for b in wave for h2 in range(2)}
                for t in range(9):
                    conv_tap(pysn, wave, t)
                    if t == 2:
                        out_proj(prev[0], pso)
                    elif t == 5:
                        out_proj(prev[1], pso)
                if wave is WAVES[-1]:
                    for b in wave:
                        for h2 in range(2):
                            nc.scalar.activation(
                                y_bs[b][:DIM, h2 * 392:(h2 + 1) * 392],
                                pysn[(b, h2)][:], AF.Gelu_apprx_sigmoid)
                        out_proj(b, pso)
                else:
                    gelu_wave(pysn, wave)
                pywn_ctx.__exit__(None, None, None)
                prev = wave
            pso_ctx.__exit__(None, None, None)

    nc.compile()
    return nc


# host-side scale shared between prep and build (prep runs first in _run and
# bakes the power-of-two Wh scale into the hbar activation scale).
_HBAR_SCALE = [1.0 / P]

_NC_CACHE = None


def _get_nc():
    global _NC_CACHE
    if _NC_CACHE is None:
        _NC_CACHE = build_nc()
    return _NC_CACHE


def _prep_inputs(x, Wd, bd, Wm1, bm1, Wm2, bm2, Wh, bh, emb, Wu, bu):
    """Host-side prep: permute/augment/quantize weights, transpose+shard x."""
    # Wh columns permuted to (tap, o, i) order, tap-major for the conv overlap
    whp = np.asarray(Wh, np.float32).reshape(E, DIM, DIM, 9)      # (e, o, i, t)
    whp = whp.transpose(0, 3, 1, 2).reshape(E, WH_COLS)           # (e, (t, o, i))
    bhp = np.asarray(bh, np.float32).reshape(DIM, DIM, 9)
    bhp = bhp.transpose(2, 0, 1).reshape(WH_COLS)                 # (t, o, i)
    cvec = (np.asarray(emb, np.float32) + np.asarray(bm2, np.float32)) @ whp + bhp

    s_wh = _pow2_scale(whp)
    wh_q = (whp * s_wh).astype(FP8_NP)
    s_hi = _pow2_scale(cvec)
    c_hi = (cvec * s_hi).astype(FP8_NP)
    resid = cvec - c_hi.astype(np.float32) / s_hi
    s_lo = _pow2_scale(resid)
    c_lo = (resid * s_lo).astype(FP8_NP)
    wh_aug = np.concatenate([wh_q, c_hi[None], c_lo[None]], 0)    # [66, WH_COLS]
    _HBAR_SCALE[0] = 1.0 / (P * s_wh)

    wu_aug = np.concatenate([np.asarray(Wu, np.float32),
                             np.asarray(bu, np.float32)[None]], 0)  # [97, C]
    shared = {
        "wm1": np.asarray(Wm1, np.float32).astype(ml_dtypes.bfloat16),
        "wd": np.asarray(Wd, np.float32).astype(ml_dtypes.bfloat16),
        "wm2": _round_fp32r(Wm2),
        "wh": np.ascontiguousarray(wh_aug),
        "wu": wu_aug.astype(ml_dtypes.bfloat16),
        "bm1": np.ascontiguousarray(bm1, np.float32),
        "bd": np.ascontiguousarray(bd, np.float32),
        "scrow": np.array([[1.0 / s_hi], [1.0 / s_lo]], np.float32),
    }
    xs = np.asarray(x, np.float32).reshape(B, P, C)
    in_maps = []
    for k in range(NCORES):
        m = dict(shared)
        xt = xs[k * BL:(k + 1) * BL].reshape(NPOS, C).T           # [C, NPOS]
        xt = xt.reshape(C // 128, 128, NPOS).transpose(1, 0, 2)   # [128, 3, NPOS]
        m["x"] = np.ascontiguousarray(xt.astype(ml_dtypes.bfloat16))
        in_maps.append(m)
    return in_maps


def _run(inputs, **spmd_kwargs):
    in_maps = _prep_inputs(**inputs)
    nc = _get_nc()
    res = run_bass_kernel_spmd(nc, in_maps, core_ids=list(range(NCORES)), **spmd_kwargs)
    out = np.concatenate([r["out"].astype(np.float32) for r in res.results], 0)
    return out.reshape(B, H, W, C), res


def kernel(**inputs) -> np.ndarray:
    out, _ = _run(inputs)
    return out


# revision 63
# speedup vs baseline: 1.0077x; 1.0077x over previous
"""Trainium2 Bass kernel for Convpass-swin hypernet fused adapter.

Reference computation (per batch sample):
  h      = relu(x @ Wm1 + bm1)                    # [B,H,W,64]
  prompt = mean_hw(h) @ Wm2                       # [B,64]  (mean commutes with matmul)
  wflat  = (emb + bm2 + prompt) @ Wh + bh         # [B,96*96*9]
  xd     = quickgelu(x @ Wd + bd)                 # [B,H,W,96]
  y      = quickgelu(conv3x3(xd, wflat))          # per-sample dynamic grouped conv
  out    = y @ Wu + bu                            # [B,H,W,384]

Sharding: data-parallel over batch B=64 across 8 cores (8 samples/core),
weights replicated.

Key layout/precision choices:
- x is shipped pre-transposed ([C, pos] per core) in bf16: no on-device
  transposes, half the DMA bytes.
- The 21MB hypernet matrix Wh streams as fp8(e3m4), scaled by a power of
  two into fp8 range.  The constant part of the conv weights,
  cvec = (emb+bm2)@Wh + bh, is folded in host-side as two extra scaled fp8
  rows (hi + lo residual), so each per-sample conv weight materializes in
  PSUM from a single matmul per 96-column group.
- The 3x3 conv is evaluated tap-major (Wh columns pre-permuted to
  (tap, o, i) order) so the first conv wave overlaps the Wh DMA stream,
  with a one-tap lag so the PSUM->SBUF weight copies never block the PE.
- Conv waves of 2 samples; each wave's matmuls are interleaved with the
  previous wave's output projection so ACT/DVE PSUM-evacuation overlaps
  PE compute.  Output is written bf16 and upcast on the host.

NOTE on DMA ordering: the wu (97-partition bf16) load must not be issued
between the other small weight loads and the x loads — that ordering
triggers a data-corruption bug in the NEFF simulation path (even bf16
elements of later transfers read back as sign*2.0).  wu is issued last
among the constant loads.
"""
import numpy as np
import ml_dtypes

import concourse.bass as bass
import concourse.tile as tile
import concourse.mybir as mybir
from concourse import bacc
from concourse.bass_utils import run_bass_kernel_spmd

F32 = mybir.dt.float32
F32R = mybir.dt.float32r
BF = mybir.dt.bfloat16
FP8 = mybir.dt.float8e3          # e3m4
FP8_NP = ml_dtypes.float8_e3m4
AF = mybir.ActivationFunctionType
AX = mybir.AxisListType
ALU = mybir.AluOpType

# problem constants
B, H, W, C = 64, 28, 28, 384
DIM, E, KK = 96, 64, 3
NCORES = 8
BL = B // NCORES          # samples per core
P = H * W                 # 784 positions per sample
HP = H + 2                # padded spatial
NPOS = BL * P             # 6272 positions per core
NG = DIM * 9              # 864 (tap, o) groups, tap-major
WH_COLS = NG * DIM        # 82944
CHUNK_G = 48              # groups per Wh DMA chunk (= half a tap)
CHUNK_COLS = CHUNK_G * DIM
NCHUNK = NG // CHUNK_G    # 18
WAVES = ((0, 1), (2, 3), (4, 5), (6, 7))


def _round_fp32r(a):
    """Round-to-nearest-even fp32 -> fp32r (11-bit mantissa kept)."""
    b = np.ascontiguousarray(a, np.float32).view(np.uint32).astype(np.uint64)
    bb = b + np.uint64(0x7FF) + ((b >> np.uint64(12)) & np.uint64(1))
    return (bb & np.uint64(0xFFFFF000)).astype(np.uint32).view(np.float32)


def _pow2_scale(a, target=8.0):
    """Power-of-two S with absmax(a)*S <= target."""
    m = float(np.abs(a).max())
    if m == 0.0:
        return 1.0
    return 2.0 ** np.floor(np.log2(target / m))


def build_nc():
    nc = bacc.Bacc("TRN2", target_bir_lowering=False, debug=False)

    x_d = nc.dram_tensor("x", [128, C // 128, NPOS], BF, kind="ExternalInput").ap()
    wm1_d = nc.dram_tensor("wm1", [C, E], BF, kind="ExternalInput").ap()
    wd_d = nc.dram_tensor("wd", [C, DIM], BF, kind="ExternalInput").ap()
    wm2_d = nc.dram_tensor("wm2", [E, E], F32R, kind="ExternalInput").ap()
    bm1_d = nc.dram_tensor("bm1", [E], F32, kind="ExternalInput").ap()
    bd_d = nc.dram_tensor("bd", [DIM], F32, kind="ExternalInput").ap()
    scrow_d = nc.dram_tensor("scrow", [2, 1], F32, kind="ExternalInput").ap()
    wh_d = nc.dram_tensor("wh", [E + 2, WH_COLS], FP8, kind="ExternalInput").ap()
    wu_d = nc.dram_tensor("wu", [DIM + 1, C], BF, kind="ExternalInput").ap()
    out_d = nc.dram_tensor("out", [NPOS, C], BF, kind="ExternalOutput").ap()

    with tile.TileContext(nc) as tc:
        with (
            tc.tile_pool(name="const", bufs=1) as cp,
            tc.tile_pool(name="persist", bufs=1) as pp,
            tc.tile_pool(name="wh", bufs=10) as wh_p,
            tc.tile_pool(name="hscr", bufs=2) as hs_p,
            tc.tile_pool(name="outp", bufs=6) as out_p,
        ):
            # ---- weight loads needed first, then x, then the rest ----
            wm1_sb = cp.tile([128, C // 128, E], BF)
            nc.sync.dma_start(wm1_sb[:], wm1_d.rearrange("(c3 p) e -> p c3 e", p=128))
            wd_sb = cp.tile([128, C // 128, DIM], BF)
            nc.sync.dma_start(wd_sb[:], wd_d.rearrange("(c3 p) e -> p c3 e", p=128))
            wm2_sb = cp.tile([E, E], F32R)
            nc.sync.dma_start(wm2_sb[:], wm2_d[:])
            bm1_sb = cp.tile([E, 1], F32)
            nc.sync.dma_start(bm1_sb[:], bm1_d[:])
            bd_sb = cp.tile([DIM, 1], F32)
            nc.sync.dma_start(bd_sb[:], bd_d[:])
            scrow_sb = cp.tile([2, 1], F32)
            nc.sync.dma_start(scrow_sb[:], scrow_d[:])
            wu_sb = cp.tile([DIM + 1, C], BF)
            nc.sync.dma_start(wu_sb[:], wu_d[:])

            xts = [pp.tile([128, C // 128, P], BF, name=f"xt{b}") for b in range(BL)]
            nc.sync.dma_start(xts[0][:, :, :392], x_d[:, :, 0:392])
            nc.sync.dma_start(xts[0][:, :, 392:], x_d[:, :, 392:P])
            for b in range(1, BL):
                nc.sync.dma_start(xts[b][:], x_d[:, :, b * P:(b + 1) * P])

            # ---- persistent state ----
            xd_pads = [pp.tile([DIM, HP, HP], BF, name=f"xdp{b}") for b in range(BL)]
            w_ts = [pp.tile([DIM, BL, DIM], BF, name=f"wt{t}") for t in range(9)]
            y_bs = [pp.tile([DIM + 1, P], BF, name=f"y{b}") for b in range(BL)]
            hsum = pp.tile([E, BL, 2], F32)
            hbar = pp.tile([E, BL], F32)
            hbar_r = pp.tile([E, BL], F32R)
            pvec = pp.tile([E + 2, BL], BF)

            # Pool-engine inits: conv-pad borders, y ones rows
            for b in range(BL):
                xp = xd_pads[b]
                nc.gpsimd.memset(xp[:, 0:1, :], 0.0)
                nc.gpsimd.memset(xp[:, HP - 1:HP, :], 0.0)
                nc.gpsimd.memset(xp[:, 1:HP - 1, 0:1], 0.0)
                nc.gpsimd.memset(xp[:, 1:HP - 1, HP - 1:HP], 0.0)
                nc.gpsimd.memset(y_bs[b][DIM:DIM + 1, :], 1.0)
            nc.vector.tensor_copy(pvec[E:E + 2, :],
                                  scrow_sb[:, 0:1].to_broadcast((2, BL)))

            # ---- phase 1: per-sample meta sums + xd = gelu(x@Wd + bd) ----
            ps1 = tc.tile_pool(name="ps1", bufs=2, space="PSUM")
            ps1_pool = ps1.__enter__()
            for b in range(BL):
                for h2 in range(2):
                    ph = ps1_pool.tile([E, 392], F32, name="ph", tag="ph")
                    for c in range(C // 128):
                        nc.tensor.matmul(ph[:], wm1_sb[:, c, :],
                                         xts[b][:, c, h2 * 392:(h2 + 1) * 392],
                                         start=(c == 0), stop=(c == 2))
                    if h2 == 0:
                        h_scr = hs_p.tile([E, 392], F32, tag="hscr")
                        nc.scalar.activation(h_scr[:], ph[:], AF.Relu, bias=bm1_sb[:],
                                             accum_out=hsum[:, b, 0:1])
                    else:
                        h_scr = hs_p.tile([E, 392], F32, tag="hscr")
                        nc.vector.tensor_scalar(h_scr[:], ph[:], bm1_sb[:, 0:1], 0.0,
                                                ALU.add, ALU.max)
                        nc.vector.reduce_sum(hsum[:, b, 1:2], h_scr[:], axis=AX.X)
                for h2 in range(2):
                    px = ps1_pool.tile([DIM, 392], F32, name="px", tag="px")
                    for c in range(C // 128):
                        nc.tensor.matmul(px[:], wd_sb[:, c, :],
                                         xts[b][:, c, h2 * 392:(h2 + 1) * 392],
                                         start=(c == 0), stop=(c == 2))
                    nc.scalar.activation(
                        xd_pads[b][:, 1 + h2 * 14: 15 + h2 * 14, 1:29],
                        px[:].rearrange("p (r c) -> p r c", r=14),
                        AF.Gelu_apprx_sigmoid, bias=bd_sb[:])

            # ---- phase 2: prompt -> pvec (scaled by 1/(P*S_wh)) ----
            nc.vector.reduce_sum(hbar[:], hsum[:], axis=AX.X)
            nc.scalar.activation(hbar_r[:], hbar[:], AF.Copy, scale=_HBAR_SCALE[0])
            ppm = ps1_pool.tile([E, BL], F32, name="ppm", tag="ppm", bufs=1)
            nc.tensor.matmul(ppm[:], wm2_sb[:], hbar_r[:], start=True, stop=True)
            nc.scalar.activation(pvec[:E, :], ppm[:], AF.Copy)
            ps1.__exit__(None, None, None)

            def conv_tap(pys_map, wave, t):
                dy, dx = t // 3, t % 3
                for b in wave:
                    for h2 in range(2):
                        nc.tensor.matmul(
                            pys_map[(b, h2)][:], w_ts[t][:, b, :],
                            xd_pads[b][:, h2 * 14 + dy: h2 * 14 + dy + 14,
                                       dx:dx + 28],
                            start=(t == 0), stop=(t == 8))

            def gelu_wave(pys_map, wave):
                for b in wave:
                    for h2 in range(2):
                        nc.scalar.activation(y_bs[b][:DIM, h2 * 392:(h2 + 1) * 392],
                                             pys_map[(b, h2)][:],
                                             AF.Gelu_apprx_sigmoid)

            def out_proj(b, pso):
                ob = out_p.tile([112, 7, C], BF, tag="ob")
                odst = out_d[b * P:(b + 1) * P, :].rearrange("(m p) c -> p m c", p=112)
                for m in range(7):
                    po = pso.tile([112, C], F32, name="po", tag="po")
                    nc.tensor.matmul(po[:], y_bs[b][:, m * 112:(m + 1) * 112], wu_sb[:],
                                     start=True, stop=True)
                    if m % 2 == 0:
                        nc.vector.tensor_copy(ob[:, m, :], po[:])
                    else:
                        nc.scalar.activation(ob[:, m, :], po[:], AF.Copy)
                    if m == 3:
                        nc.sync.dma_start(odst[:, 0:4, :], ob[:, 0:4, :])
                nc.sync.dma_start(odst[:, 4:7, :], ob[:, 4:7, :])

            # ---- phase 3: hypernet streaming + conv wave 0 (one-tap lag) ----
            pyw_ctx = tc.tile_pool(name="pyw0", bufs=1, space="PSUM")
            pyw = pyw_ctx.__enter__()
            pys = {(b, h2): pyw.tile([DIM, 392], F32, name=f"py{b}_{h2}")
                   for b in WAVES[0] for h2 in range(2)}
            psw_ctx = tc.tile_pool(name="psw", bufs=4, space="PSUM")
            psw = psw_ctx.__enter__()
            for j in range(NCHUNK):
                whc = wh_p.tile([E + 2, CHUNK_COLS], FP8, tag="whc")
                nc.sync.dma_start(whc[:], wh_d[:, j * CHUNK_COLS:(j + 1) * CHUNK_COLS])
                t, ohalf = j // 2, j % 2
                pwg = psw.tile([DIM, CHUNK_G, BL], F32, name="pwg", tag="pwg")
                for g in range(CHUNK_G):
                    col = g * DIM
                    nc.tensor.matmul(pwg[:, g, :], whc[:, col:col + DIM], pvec[:],
                                     start=True, stop=True)
                o0 = ohalf * CHUNK_G
                dst = w_ts[t][:, :, o0:o0 + CHUNK_G]
                srcv = pwg[:].rearrange("i g b -> i b g")
                if j % 2 == 0:
                    nc.vector.tensor_copy(dst, srcv)
                else:
                    nc.scalar.activation(dst, srcv, AF.Copy)
                if j >= 3 and j % 2 == 1:
                    conv_tap(pys, WAVES[0], (j - 3) // 2)
            conv_tap(pys, WAVES[0], 8)
            psw_ctx.__exit__(None, None, None)
            gelu_wave(pys, WAVES[0])
            pyw_ctx.__exit__(None, None, None)

            # ---- phases 4+5: conv waves 1..3 overlapped with out-projection ----
            pso_ctx = tc.tile_pool(name="pso", bufs=4, space="PSUM")
            pso = pso_ctx.__enter__()
            prev = WAVES[0]
            for wave in WAVES[1:]:
                pywn_ctx = tc.tile_pool(name=f"pyw{wave[0]}", bufs=1, space="PSUM")
                pywn = pywn_ctx.__enter__()
                pysn = {(b, h2): pywn.tile([DIM, 392], F32, name=f"py{b}_{h2}")
                        for b in wave for h2 in range(2)}
                for t in range(9):
                    conv_tap(pysn, wave, t)
                    if t == 2:
                        out_proj(prev[0], pso)
                    elif t == 5:
                        out_proj(prev[1], pso)
                if wave is WAVES[-1]:
                    for b in wave:
                        for h2 in range(2):
                            nc.scalar.activation(
                                y_bs[b][:DIM, h2 * 392:(h2 + 1) * 392],
                                pysn[(b, h2)][:], AF.Gelu_apprx_sigmoid)
                        out_proj(b, pso)
                else:
                    gelu_wave(pysn, wave)
                pywn_ctx.__exit__(None, None, None)
                prev = wave
            pso_ctx.__exit__(None, None, None)

    nc.compile()
    return nc


# host-side scale shared between prep and build (prep runs first in _run and
# bakes the power-of-two Wh scale into the hbar activation scale).
_HBAR_SCALE = [1.0 / P]

_NC_CACHE = None


def _get_nc():
    global _NC_CACHE
    if _NC_CACHE is None:
        _NC_CACHE = build_nc()
    return _NC_CACHE


def _prep_inputs(x, Wd, bd, Wm1, bm1, Wm2, bm2, Wh, bh, emb, Wu, bu):
    """Host-side prep: permute/augment/quantize weights, transpose+shard x."""
    # Wh columns permuted to (tap, o, i) order, tap-major for the conv overlap
    whp = np.asarray(Wh, np.float32).reshape(E, DIM, DIM, 9)      # (e, o, i, t)
    whp = whp.transpose(0, 3, 1, 2).reshape(E, WH_COLS)           # (e, (t, o, i))
    bhp = np.asarray(bh, np.float32).reshape(DIM, DIM, 9)
    bhp = bhp.transpose(2, 0, 1).reshape(WH_COLS)                 # (t, o, i)
    cvec = (np.asarray(emb, np.float32) + np.asarray(bm2, np.float32)) @ whp + bhp

    s_wh = _pow2_scale(whp)
    wh_q = (whp * s_wh).astype(FP8_NP)
    s_hi = _pow2_scale(cvec)
    c_hi = (cvec * s_hi).astype(FP8_NP)
    resid = cvec - c_hi.astype(np.float32) / s_hi
    s_lo = _pow2_scale(resid)
    c_lo = (resid * s_lo).astype(FP8_NP)
    wh_aug = np.concatenate([wh_q, c_hi[None], c_lo[None]], 0)    # [66, WH_COLS]
    _HBAR_SCALE[0] = 1.0 / (P * s_wh)

    wu_aug = np.concatenate([np.asarray(Wu, np.float32),
                             np.asarray(bu, np.float32)[None]], 0)  # [97, C]
    shared = {
        "wm1": np.asarray(Wm1, np.float32).astype(ml_dtypes.bfloat16),
        "wd": np.asarray(Wd, np.float32).astype(ml_dtypes.bfloat16),
        "wm2": _round_fp32r(Wm2),
        "wh": np.ascontiguousarray(wh_aug),
        "wu": wu_aug.astype(ml_dtypes.bfloat16),
        "bm1": np.ascontiguousarray(bm1, np.float32),
        "bd": np.ascontiguousarray(bd, np.float32),
        "scrow": np.array([[1.0 / s_hi], [1.0 / s_lo]], np.float32),
    }
    xs = np.asarray(x, np.float32).reshape(B, P, C)
    in_maps = []
    for k in range(NCORES):
        m = dict(shared)
        xt = xs[k * BL:(k + 1) * BL].reshape(NPOS, C).T           # [C, NPOS]
        xt = xt.reshape(C // 128, 128, NPOS).transpose(1, 0, 2)   # [128, 3, NPOS]
        m["x"] = np.ascontiguousarray(xt.astype(ml_dtypes.bfloat16))
        in_maps.append(m)
    return in_maps


def _run(inputs, **spmd_kwargs):
    in_maps = _prep_inputs(**inputs)
    nc = _get_nc()
    res = run_bass_kernel_spmd(nc, in_maps, core_ids=list(range(NCORES)), **spmd_kwargs)
    out = np.concatenate([r["out"].astype(np.float32) for r in res.results], 0)
    return out.reshape(B, H, W, C), res


def kernel(**inputs) -> np.ndarray:
    out, _ = _run(inputs)
    return out
